# revision 63
# baseline (speedup 1.0000x reference)
"""HSIC loss kernel for Trainium2, 8-core block-row sharded, fp8 DoubleRow.

hsic = sum(center(Kx) * center(Ky).T) / (n-1)^2 with
Kx[i,j] = exp(x_i.x_j - ||x_i||^2), Ky[j,i] = exp(y_j.y_i - ||y_j||^2)
(the reference's asymmetric "self-RBF" broadcasting).

Using trace identities (H idempotent), with A=Kx, B=Ky:
  T = S_AB - (csA.rsB)/n - (rsA.csB)/n + S_A*S_B/n^2
where S_AB = sum_ij A[i,j]B[j,i], csA = colsums(A), rsA = rowsums(A),
rsB = rowsums(B), csB = colsums(B).

Each core owns a 512-row slab of Ex[i,j] = A[i,j] and Eyt[i,j] = B[j,i].
Columns are pre-rotated per core so the slab sits at device columns
0-511; host un-rotates the column-indexed partials.

Per core:
 - PE: dots as fp8 DoubleRow matmuls (2 k-tiles/instruction, 4x bf16
   throughput). DoubleRow truncates (~ -6e-5 rel of the sum), which
   only matters where exp() is not ~0: the diagonal 128-block inside
   chunk 0. That block is computed with exact plain-fp8 matmuls into
   the same PSUM bank (hw zeroes the full bank on the first
   start=True, verified). The y-side per-column bias -||y_j||^2 rides
   in as one extra DoubleRow pass (ones-rows stationary x fp8 cascade
   rows moving). csA/rsB partition sums are near-free matmuls with the
   m-summed tile stationary and a [P,1] ones moving operand; S_AB uses
   [1,512] ones-stationary rows over the product tiles.
 - Act: exp over 2-3 bank PSUM groups; accum_out gives rsA/csB.
 - DVE: m-sums of Ex/Eyt tiles and the Ex*Eyt product tiles.
 - Loop order is group-major (all m for chunk-group 1, then group 2,
   ...) so compute follows DMA chunk arrival and column sums finish
   early. A few dummy warmup matmuls ramp the PE p-state before the
   first real tile.
Host sums the 8 partial sets and applies the final formula in f64.
"""

import sys

sys.path.insert(0, "/opt/trn_rl_repo")

import numpy as np
import ml_dtypes

P = 128
N = 4096
D = 1024
NCORES = 8
SLAB = N // NCORES        # 512 rows per core
MT = SLAB // P            # 4 m-tiles per slab
CHUNK = 512
NCH = N // CHUNK          # 8 column chunks
KT = D // P               # 8 k-tiles
KP = KT // 2              # 4 DoubleRow k-pairs
NBIAS = 8                 # fp8 cascade rows for -||y||^2
NWARM = 5                 # PE p-state warmup matmuls

# chunk groups, processed group-major (all m per group); pools ping-pong
GROUPS = [(0, 1), (2, 3, 4), (5, 6, 7)]
NG = len(GROUPS)

_compiled = {}


def _build_program():
    import concourse.bacc as bacc
    import concourse.mybir as mybir
    import concourse.tile as tile

    f32 = mybir.dt.float32
    f8 = mybir.dt.float8e4
    bf16 = mybir.dt.bfloat16
    Exp = mybir.ActivationFunctionType.Exp
    mult = mybir.AluOpType.mult
    add = mybir.AluOpType.add
    DR = mybir.MatmulPerfMode.DoubleRow

    nc = bacc.Bacc("TRN2", target_bir_lowering=False, debug=False,
                   num_devices=NCORES)

    xt = nc.dram_tensor("xt", [P, KT, N], f8, kind="ExternalInput")
    yt = nc.dram_tensor("yt", [P, KT, N], f8, kind="ExternalInput")
    sqxn = nc.dram_tensor("sqxn", [P, MT], f32, kind="ExternalInput")
    ybias = nc.dram_tensor("ybias", [P, 2, N], f8, kind="ExternalInput")
    onesr = nc.dram_tensor("onesr", [P, 2, P], f8, kind="ExternalInput")

    # packed: [0:64] csa/rsb, [64:76] rsa, [76:88] csb, [88] sab
    o_all = nc.dram_tensor("o_all", [P, 89], f32, kind="ExternalOutput")

    with tile.TileContext(nc) as tc:
        with (
            tc.tile_pool(name="big", bufs=1) as big,
            tc.tile_pool(name="eywork", bufs=3) as eywork,
            tc.tile_pool(name="scwork", bufs=3) as scwork,
            tc.tile_pool(name="pa", bufs=1, space="PSUM") as pa,
            tc.tile_pool(name="pb", bufs=1, space="PSUM") as pb,
            tc.tile_pool(name="pacc", bufs=1, space="PSUM") as pacc,
            tc.tile_pool(name="psab", bufs=1, space="PSUM") as psab,
        ):
            xt_sb = big.tile([P, KT, N], f8, tag="xt")
            yt_sb = big.tile([P, KT, N], f8, tag="yt")
            sqx_sb = big.tile([P, MT], f32, tag="sq")
            yb_sb = big.tile([P, 2, N], f8, tag="yb")
            onesr_sb = big.tile([P, 2, P], f8, tag="onesr")
            ones1 = big.tile([P, 1], bf16, tag="ones1")
            warm_sb = big.tile([P, CHUNK], bf16, tag="warm")
            eyt_sb = big.tile([P, MT, NCH, CHUNK], bf16, tag="eyt")
            sumx_sb = big.tile([P, NCH, CHUNK], bf16, tag="sumx")
            sumy_sb = big.tile([P, NCH, CHUNK], bf16, tag="sumy")
            out_sb = big.tile([P, 89], f32, tag="out")

            acc_ps = pacc.tile([P, 64], f32, tag="acc")
            sab_ps = psab.tile([1, CHUNK], f32, tag="sabps")
            pools = (pa, pb)

            def xpair(c):
                return (xt_sb[:, :, c * CHUNK:(c + 2) * CHUNK],
                        xt[:, :, c * CHUNK:(c + 2) * CHUNK])

            def ypair(c):
                return (yt_sb[:, :, c * CHUNK:(c + 2) * CHUNK],
                        yt[:, :, c * CHUNK:(c + 2) * CHUNK])

            # input DMAs split over the SP and Pool queues, ordered by the
            # time compute first needs each piece; the y side runs first,
            # chunk 0 and the first bias slice ride alone for an early start
            nc.sync.dma_start(yt_sb[:, :, 0:CHUNK], yt[:, :, 0:CHUNK])
            nc.gpsimd.dma_start(yb_sb[:, :, 0:2 * CHUNK],
                                ybias[:, :, 0:2 * CHUNK])
            nc.gpsimd.dma_start(onesr_sb[:], onesr[:])
            nc.sync.dma_start(yt_sb[:, :, CHUNK:2 * CHUNK],
                              yt[:, :, CHUNK:2 * CHUNK])
            nc.gpsimd.dma_start(*ypair(2))
            nc.sync.dma_start(*ypair(4))
            nc.gpsimd.dma_start(yb_sb[:, :, 2 * CHUNK:N],
                                ybias[:, :, 2 * CHUNK:N])
            nc.sync.dma_start(*ypair(6))
            nc.gpsimd.dma_start(*xpair(0))
            nc.sync.dma_start(sqx_sb[:], sqxn[:])
            nc.gpsimd.dma_start(*xpair(2))
            nc.sync.dma_start(*xpair(4))
            nc.gpsimd.dma_start(*xpair(6))
            nc.vector.memset(ones1[:], 1.0)
            nc.vector.memset(warm_sb[:], 0.0)
            nc.vector.memset(acc_ps[:], 0.0)

            # PE p-state warmup: dummy rows into sab_ps keep the PE busy
            # through the ramp window before the first real tile
            for i in range(NWARM):
                nc.tensor.matmul(sab_ps[:], ones1[:], warm_sb[:],
                                 start=True, stop=True,
                                 skip_group_check=True)
            nc.vector.memset(sab_ps[:], 0.0)

            def nondiag_runs(m):
                runs = []
                if m > 0:
                    runs.append((0, m * P))
                if (m + 1) * P < CHUNK:
                    runs.append(((m + 1) * P, CHUNK))
                return runs

            def emit_x_tile(ps, ci, c, m, sl):
                cs = slice(c * CHUNK, (c + 1) * CHUNK)
                if c == 0:
                    # exact diagonal block; k0's start=True zeroes the bank
                    for k in range(KT):
                        nc.tensor.matmul(
                            ps[:, ci, sl], xt_sb[:, k, sl], xt_sb[:, k, sl],
                            start=(k == 0), stop=False,
                            skip_group_check=True)
                    runs = nondiag_runs(m)
                    for ri, (a, b) in enumerate(runs):
                        lastr = ri == len(runs) - 1
                        for kp in range(KP):
                            nc.tensor.matmul(
                                ps[:, ci, a:b],
                                xt_sb[:, 2 * kp:2 * kp + 2, sl],
                                xt_sb[:, 2 * kp:2 * kp + 2, a:b],
                                start=False,
                                stop=(lastr and kp == KP - 1),
                                perf_mode=DR, skip_group_check=True)
                else:
                    for kp in range(KP):
                        nc.tensor.matmul(
                            ps[:, ci],
                            xt_sb[:, 2 * kp:2 * kp + 2, sl],
                            xt_sb[:, 2 * kp:2 * kp + 2, cs],
                            start=(kp == 0), stop=(kp == KP - 1),
                            perf_mode=DR)

            def emit_y_tile(ps, ci, c, m, sl):
                cs = slice(c * CHUNK, (c + 1) * CHUNK)
                if c == 0:
                    for k in range(KT):
                        nc.tensor.matmul(
                            ps[:, ci, sl], yt_sb[:, k, sl], yt_sb[:, k, sl],
                            start=(k == 0), stop=False,
                            skip_group_check=True)
                    nc.tensor.matmul(
                        ps[:, ci, sl], onesr_sb[:, 0], yb_sb[:, 0, sl],
                        start=False, stop=False, skip_group_check=True)
                    runs = nondiag_runs(m)
                    for ri, (a, b) in enumerate(runs):
                        lastr = ri == len(runs) - 1
                        for kp in range(KP):
                            nc.tensor.matmul(
                                ps[:, ci, a:b],
                                yt_sb[:, 2 * kp:2 * kp + 2, sl],
                                yt_sb[:, 2 * kp:2 * kp + 2, a:b],
                                start=False, stop=False,
                                perf_mode=DR, skip_group_check=True)
                        nc.tensor.matmul(
                            ps[:, ci, a:b], onesr_sb[:],
                            yb_sb[:, :, a:b],
                            start=False, stop=(lastr),
                            perf_mode=DR, skip_group_check=True)
                else:
                    for kp in range(KP):
                        nc.tensor.matmul(
                            ps[:, ci],
                            yt_sb[:, 2 * kp:2 * kp + 2, sl],
                            yt_sb[:, 2 * kp:2 * kp + 2, cs],
                            start=(kp == 0), stop=False,
                            perf_mode=DR)
                    nc.tensor.matmul(
                        ps[:, ci], onesr_sb[:], yb_sb[:, :, cs],
                        start=False, stop=True, perf_mode=DR)

            # ---- compute, group-major blocks; y first (Eyt persists), the
            # products + S_AB ride in the x phase, whose other PE load is
            # lighter
            def emit_y_block(gi):
                chunks = GROUPS[gi]
                gl = len(chunks)
                c0 = chunks[0]
                for m in range(MT):
                    sl = slice(m * P, (m + 1) * P)
                    ps = pools[m % 2].tile([P, 3, CHUNK], f32,
                                           tag=f"ps{m % 2}")
                    for ci, c in enumerate(chunks):
                        emit_y_tile(ps, ci, c, m, sl)
                    g = m * NG + gi
                    nc.scalar.activation(
                        eyt_sb[:, m, c0:c0 + gl], ps[:, 0:gl], Exp,
                        accum_out=out_sb[:, 76 + g:77 + g],
                    )
                    if m == 0:
                        nc.vector.tensor_copy(
                            sumy_sb[:, c0:c0 + gl], eyt_sb[:, 0, c0:c0 + gl])
                    else:
                        nc.vector.tensor_tensor(
                            sumy_sb[:, c0:c0 + gl], sumy_sb[:, c0:c0 + gl],
                            eyt_sb[:, m, c0:c0 + gl], add)
                # rsB partials for this group's chunks
                for c in chunks:
                    for q in range(4):
                        nc.tensor.matmul(
                            acc_ps[:, 32 + c * 4 + q:32 + c * 4 + q + 1],
                            sumy_sb[:, c, q * P:(q + 1) * P], ones1[:],
                            start=False, stop=True, skip_group_check=True)

            def emit_x_block(gi):
                chunks = GROUPS[gi]
                gl = len(chunks)
                c0 = chunks[0]
                for m in range(MT):
                    sl = slice(m * P, (m + 1) * P)
                    ps = pools[m % 2].tile([P, 3, CHUNK], f32,
                                           tag=f"ps{m % 2}")
                    for ci, c in enumerate(chunks):
                        emit_x_tile(ps, ci, c, m, sl)
                    g = m * NG + gi
                    ext = eywork.tile([P, 3, CHUNK], bf16, tag="ext")
                    nc.scalar.activation(
                        ext[:, 0:gl], ps[:, 0:gl], Exp,
                        bias=sqx_sb[:, m:m + 1],
                        accum_out=out_sb[:, 64 + g:65 + g],
                    )
                    scr = scwork.tile([P, 3, CHUNK], bf16, tag="scr")
                    nc.vector.tensor_tensor(
                        scr[:, 0:gl], eyt_sb[:, m, c0:c0 + gl],
                        ext[:, 0:gl], mult)
                    # S_AB partials: [1,512] rows over all product tiles
                    for ci, c in enumerate(chunks):
                        nc.tensor.matmul(
                            sab_ps[:], ones1[:], scr[:, ci],
                            start=False,
                            stop=(gi == NG - 1 and m == MT - 1
                                  and ci == gl - 1),
                            skip_group_check=True)
                    if m == 0:
                        nc.vector.tensor_copy(
                            sumx_sb[:, c0:c0 + gl], ext[:, 0:gl])
                    else:
                        nc.vector.tensor_tensor(
                            sumx_sb[:, c0:c0 + gl], sumx_sb[:, c0:c0 + gl],
                            ext[:, 0:gl], add)
                # csA partials for this group's chunks
                for c in chunks:
                    for q in range(4):
                        nc.tensor.matmul(
                            acc_ps[:, c * 4 + q:c * 4 + q + 1],
                            sumx_sb[:, c, q * P:(q + 1) * P], ones1[:],
                            start=False, stop=True, skip_group_check=True)

            emit_y_block(0)
            emit_y_block(1)
            emit_y_block(2)
            emit_x_block(0)
            emit_x_block(1)
            emit_x_block(2)

            nc.any.tensor_copy(out_sb[:, 0:64], acc_ps[:])
            nc.vector.tensor_reduce(out_sb[0:1, 88:89], sab_ps[:],
                                    mybir.AxisListType.X, add)
            nc.sync.dma_start(o_all[:], out_sb[:])

    nc.compile()
    return nc


def _get_program():
    if "nc" not in _compiled:
        _compiled["nc"] = _build_program()
    return _compiled["nc"]


def _f8cast(a):
    return a.astype(ml_dtypes.float8_e4m3)


def prepare_in_maps(x: np.ndarray, y: np.ndarray):
    """Host-side fp8 quantize + layout prep. Columns are rotated per core
    so each core's slab sits at device columns 0-511."""
    x8 = _f8cast(np.asarray(x, dtype=np.float32))
    y8 = _f8cast(np.asarray(y, dtype=np.float32))

    # norms of the quantized values the device actually dots
    sqx = (x8.astype(np.float32) ** 2).sum(axis=1)     # [N]
    sqy = (y8.astype(np.float32) ** 2).sum(axis=1)

    # [P, KT, N]: xtr[p, k, j] = x8[j, k*128+p]
    xtr = np.ascontiguousarray(
        x8.T.reshape(KT, P, N).transpose(1, 0, 2))
    ytr = np.ascontiguousarray(
        y8.T.reshape(KT, P, N).transpose(1, 0, 2))

    # fp8 cascade rows summing to -sqy (error ~1e-3)
    r = (-sqy).astype(np.float64)
    ybias = np.zeros((P, 2, N), dtype=ml_dtypes.float8_e4m3)
    for i in range(NBIAS):
        h = _f8cast(np.clip(r, -240.0, 240.0).astype(np.float32))
        ybias[i, 0, :] = h
        r = r - h.astype(np.float64)

    onesr = np.zeros((P, 2, P), dtype=ml_dtypes.float8_e4m3)
    onesr[0:NBIAS, 0, :] = 1.0

    in_maps = []
    for d in range(NCORES):
        sl = slice(d * SLAB, (d + 1) * SLAB)
        sh = -d * SLAB
        in_maps.append({
            "xt": np.ascontiguousarray(np.roll(xtr, sh, axis=2)),
            "yt": np.ascontiguousarray(np.roll(ytr, sh, axis=2)),
            "sqxn": np.ascontiguousarray((-sqx[sl]).reshape(MT, P).T),
            "ybias": np.ascontiguousarray(np.roll(ybias, sh, axis=2)),
            "onesr": onesr,
        })
    return in_maps


def combine_results(results):
    """Sum per-core partials and apply the final HSIC formula (host, f64)."""
    n = float(N)
    csa = np.zeros(N, dtype=np.float64)
    rsb = np.zeros(N, dtype=np.float64)
    rsa = np.zeros(N, dtype=np.float64)
    csb = np.zeros(N, dtype=np.float64)
    s_ab = 0.0
    for d, r in enumerate(results):
        out = r["o_all"].astype(np.float64)              # [P, 89]
        # col t=c*4+q, partition p -> device column j' = c*512 + q*128 + p;
        # global column j = (d*512 + j') % N  (columns were pre-rotated)
        csa += np.roll(
            out[:, 0:32].reshape(P, NCH, 4).transpose(1, 2, 0).ravel(),
            d * SLAB)
        rsb += np.roll(
            out[:, 32:64].reshape(P, NCH, 4).transpose(1, 2, 0).ravel(),
            d * SLAB)
        s_ab += out[0, 88]
        sl = slice(d * SLAB, (d + 1) * SLAB)
        # [P, MT*NG] -> sum groups -> row i = m*128 + p within the slab
        rsa[sl] = out[:, 64:64 + MT * NG].reshape(P, MT, NG).sum(
            axis=2).T.ravel()
        csb[sl] = out[:, 76:76 + MT * NG].reshape(P, MT, NG).sum(
            axis=2).T.ravel()
    s_a = csa.sum()
    s_b = rsb.sum()
    t = s_ab - (csa @ rsb) / n - (rsa @ csb) / n + s_a * s_b / (n * n)
    return np.float32(t / ((n - 1.0) ** 2))


def kernel(x: np.ndarray, y: np.ndarray) -> np.ndarray:
    from concourse.bass_utils import run_bass_kernel_spmd

    nc = _get_program()
    in_maps = prepare_in_maps(np.asarray(x), np.asarray(y))
    res = run_bass_kernel_spmd(nc, in_maps, core_ids=list(range(NCORES)))
    return combine_results(res.results)
